# revision 31
# baseline (speedup 1.0000x reference)
"""Trainium2 Bass kernel for nn_Attention (B=2, S=2048, D=1024, H=16).

Sharding: tensor-parallel over heads. Each of the 8 cores owns 2 heads
(both batches): it computes q,k,v projections for its head columns, full
attention for its 4 (batch, head) pairs, and a partial output projection
(contraction over its 128 head-output columns). The host sums the 8
partials and adds b_proj.

Performance notes (measured on TRN2):
 - The PE HAM clock-gate only holds 2.4 GHz while "full" matmuls keep
   coming; the attention phase's half-array matmuls (K=64 scores,
   M=65 PV) alone leave it at 1.2 GHz. Stage A of the other batch and
   stage C are therefore emitted interleaved into the attention loops
   (>= ~1 full-array matmul / 1.3us keeps the clock warm).
 - exp on ScalarE has ~256ns fixed cost per op: score tiles are fused
   to [128,1024] (two keyblocks) when the additive mask is all-zero.
"""

import sys

sys.path.insert(0, "/opt/trn_rl_repo")

import numpy as np
import ml_dtypes

B, S, D, H, HD = 2, 2048, 1024, 16, 64
NCORES = 8
HPC = H // NCORES  # heads per core = 2
BS = B * S  # 4096
KB = S // 128  # key blocks per batch = 16
QT = 512  # query tile
NQT = S // QT  # query tiles per batch = 4
DC = D // 128  # contraction chunks = 8

BF16 = ml_dtypes.bfloat16

_cache = {}


def _build(zero_mask):
    import concourse.bass as bass
    import concourse.mybir as mybir
    import concourse.tile as tile
    from concourse import bacc
    from concourse.masks import make_identity

    fp32 = mybir.dt.float32
    bf16 = mybir.dt.bfloat16
    EXP = mybir.ActivationFunctionType.Exp

    nc = bacc.Bacc("TRN2", target_bir_lowering=False, debug=False,
                   num_devices=NCORES)

    xt_d = nc.dram_tensor("xt", [D, BS], bf16, kind="ExternalInput").ap()
    wq_d = nc.dram_tensor("wq", [D, 128], bf16, kind="ExternalInput").ap()
    wk_d = nc.dram_tensor("wk", [D, 128], bf16, kind="ExternalInput").ap()
    wv_d = nc.dram_tensor("wv", [D, 128], bf16, kind="ExternalInput").ap()
    bq_d = nc.dram_tensor("bq", [128, 1], fp32, kind="ExternalInput").ap()
    bk_d = nc.dram_tensor("bk", [128, 1], fp32, kind="ExternalInput").ap()
    bv_d = nc.dram_tensor("bv", [128, 1], fp32, kind="ExternalInput").ap()
    wp_d = nc.dram_tensor("wp", [128, D], bf16, kind="ExternalInput").ap()
    mk_d = nc.dram_tensor("maskt", [128, B * KB], fp32, kind="ExternalInput").ap()
    out_d = nc.dram_tensor("out", [BS, D], fp32, kind="ExternalOutput").ap()

    with tile.TileContext(nc) as tc:
        with (
            tc.tile_pool(name="const", bufs=1) as cpool,
            tc.tile_pool(name="xt", bufs=2 * DC) as xpool,
            tc.tile_pool(name="qkv", bufs=2) as qkvpool,
            tc.tile_pool(name="vp", bufs=2 * HPC * KB) as vppool,
            tc.tile_pool(name="pt", bufs=6) as ptpool,
            tc.tile_pool(name="otn", bufs=2) as otnpool,
            tc.tile_pool(name="small", bufs=2) as smpool,
            tc.tile_pool(name="cout", bufs=3) as coutpool,
            tc.tile_pool(name="ps_a", bufs=2, space="PSUM") as ps_a,
            tc.tile_pool(name="ps_st", bufs=2, space="PSUM") as ps_st,
            tc.tile_pool(name="ps_ot", bufs=2, space="PSUM") as ps_ot,
        ):
            # ---- constants ----
            wq_sb = cpool.tile([128, DC, 128], bf16)
            wk_sb = cpool.tile([128, DC, 128], bf16)
            wv_sb = cpool.tile([128, DC, 128], bf16)
            for w_sb, w_d in ((wq_sb, wq_d), (wk_sb, wk_d), (wv_sb, wv_d)):
                nc.sync.dma_start(w_sb[:], w_d.rearrange("(c p) m -> p c m", p=128))
            wp_sb = cpool.tile([128, D], bf16)
            nc.sync.dma_start(wp_sb[:], wp_d)
            bq_sb = cpool.tile([128, 1], fp32)
            bk_sb = cpool.tile([128, 1], fp32)
            bv_sb = cpool.tile([128, 1], fp32)
            for b_sb, b_d in ((bq_sb, bq_d), (bk_sb, bk_d), (bv_sb, bv_d)):
                nc.sync.dma_start(b_sb[:], b_d)
            mk_sb = cpool.tile([128, B * KB], fp32)
            nc.sync.dma_start(mk_sb[:], mk_d)
            ident = cpool.tile([128, 128], bf16)
            make_identity(nc, ident[:])

            qkvs = {}
            vps = {}
            otns = {}

            def gen_a(b):
                """Stage A for batch b: qT/kT/vT [128, S] (transposed,
                2 heads stacked). Yields after each emitted unit."""
                qT = qkvpool.tile([128, S], bf16, tag="qT", name=f"qT_{b}")
                kT = qkvpool.tile([128, S], bf16, tag="kT", name=f"kT_{b}")
                vT = qkvpool.tile([128, S], bf16, tag="vT", name=f"vT_{b}")
                qkvs[b] = (qT, kT, vT)
                for t in range(NQT):
                    xts = []
                    for c in range(DC):
                        xt = xpool.tile([128, QT], bf16, tag="xt", name="xt")
                        nc.sync.dma_start(
                            xt[:], xt_d[c * 128:(c + 1) * 128,
                                        b * S + t * QT: b * S + (t + 1) * QT])
                        xts.append(xt)
                    for (dst, w_sb, b_sb, tg) in (
                        (qT, wq_sb, bq_sb, "q"),
                        (kT, wk_sb, bk_sb, "k"),
                        (vT, wv_sb, bv_sb, "v"),
                    ):
                        a_ps = ps_a.tile([128, QT], fp32, tag="a",
                                         name=f"a_ps_{tg}")
                        for c in range(DC):
                            nc.tensor.matmul(a_ps[:], w_sb[:, c, :], xts[c][:],
                                             start=(c == 0), stop=(c == DC - 1))
                            yield
                        nc.vector.tensor_scalar_add(
                            dst[:, t * QT:(t + 1) * QT], a_ps[:], b_sb[:])
                        yield

            def gen_vtr(b):
                """v' tiles for batch b: [128 keys, 64 v + ones]."""
                vT = qkvs[b][2]
                for h in range(HPC):
                    for j in range(KB):
                        vtr_ps = ps_a.tile([128, 64], bf16, tag="a",
                                           name="vtr_ps")
                        nc.tensor.transpose(
                            vtr_ps[:],
                            vT[h * 64:(h + 1) * 64, j * 128:(j + 1) * 128],
                            ident[h * 64:(h + 1) * 64, h * 64:(h + 1) * 64])
                        vp = vppool.tile([128, 65], bf16, tag="vp",
                                         name=f"vp_{b}_{h}_{j}")
                        nc.vector.tensor_copy(vp[:, 0:64], vtr_ps[:])
                        nc.gpsimd.memset(vp[:, 64:65], 1.0)
                        vps[(b, h, j)] = vp
                        yield

            def gen_c(b, rows=None):
                """Stage C for batch b: partial out-projection."""
                otn = otns[b]
                for r in (range(S // 128) if rows is None else rows):
                    for n in range(D // QT):
                        c_ps = ps_a.tile([128, QT], fp32, tag="a", name="c_ps")
                        nc.tensor.matmul(c_ps[:],
                                         otn[:, r * 128:(r + 1) * 128],
                                         wp_sb[:, n * QT:(n + 1) * QT],
                                         start=True, stop=True)
                        co = coutpool.tile([128, QT], fp32, tag="co")
                        nc.vector.tensor_copy(co[:], c_ps[:])
                        nc.sync.dma_start(
                            out_d[b * S + r * 128: b * S + (r + 1) * 128,
                                  n * QT:(n + 1) * QT], co[:])
                        yield

            heat_sb = cpool.tile([128, QT], fp32)

            class Heater:
                """Dummy full-array accumulation chains into the ps_a ring
                (drained to a scratch sbuf tile). Keeps the PE HAM clock at
                2.4 GHz when no real full-array work is available. pull()
                emits one matmul; finish() closes any open chain."""

                def __init__(self):
                    self.tile = None
                    self.c = 0

                def pull(self):
                    if self.tile is None:
                        self.tile = ps_a.tile([128, QT], fp32, tag="a",
                                              name="heat_ps")
                        self.c = 0
                    nc.tensor.matmul(self.tile[:], wq_sb[:, self.c, :],
                                     wp_sb[:, 0:QT],
                                     start=(self.c == 0),
                                     stop=(self.c == DC - 1))
                    self.c += 1
                    if self.c == DC:
                        nc.vector.tensor_copy(heat_sb[:], self.tile[:])
                        self.tile = None

                def finish(self):
                    while self.tile is not None:
                        self.pull()

            def emit_attention(b, side, units_per_jj, post_t_side=None,
                               heater=None):
                """Attention for batch b; pulls `units_per_jj` units of
                side work after each keyblock-pair to keep the PE's HAM
                clock warm with full-array matmuls. post_t_side(t) may
                return an extra generator to append after query-tile t's
                normalization is emitted (trace-order-safe C(b1)).
                `heater` fills when real side work runs out."""
                qT, kT, vT = qkvs[b]
                otn = otnpool.tile([128, S], bf16, tag="otn", name=f"otn_{b}")
                otns[b] = otn
                for t in range(NQT):
                    ot_ps = [ps_ot.tile([65, QT], fp32, tag="ot",
                                        name=f"ot_ps_{b}_{t}_{h}")
                             for h in range(HPC)]
                    for jj in range(KB // 2):
                        j0, j1 = 2 * jj, 2 * jj + 1
                        st2s = [ps_st.tile([128, 2 * QT], fp32, tag="st",
                                           name=f"st_{h}")
                                for h in range(HPC)]
                        # both heads' K=64 score matmuls issued as an
                        # atomic pair: they occupy disjoint PE row-groups
                        # (rows 0-63 / 64-127) and run concurrently.
                        for ji, jx in ((0, j0), (1, j1)):
                            with tc.tile_critical():
                                for h in range(HPC):
                                    hs = slice(h * 64, (h + 1) * 64)
                                    nc.tensor.matmul(
                                        st2s[h][:, ji * QT:(ji + 1) * QT],
                                        kT[hs, jx * 128:(jx + 1) * 128],
                                        qT[hs, t * QT:(t + 1) * QT],
                                        start=True, stop=True)
                        for h in range(HPC):
                            st2 = st2s[h]
                            pt2 = ptpool.tile([128, 2 * QT], bf16, tag="pt",
                                              name=f"pt_{h}")
                            if zero_mask:
                                nc.scalar.activation(pt2[:], st2[:], EXP,
                                                     bias=0.0, scale=0.125)
                            else:
                                for ji, jx in ((0, j0), (1, j1)):
                                    nc.scalar.activation(
                                        pt2[:, ji * QT:(ji + 1) * QT],
                                        st2[:, ji * QT:(ji + 1) * QT], EXP,
                                        bias=mk_sb[:, b * KB + jx:
                                                   b * KB + jx + 1],
                                        scale=0.125)
                            nc.tensor.matmul(ot_ps[h][:], vps[(b, h, j0)][:],
                                             pt2[:, 0:QT],
                                             start=(jj == 0), stop=False)
                            nc.tensor.matmul(ot_ps[h][:], vps[(b, h, j1)][:],
                                             pt2[:, QT:2 * QT],
                                             start=False,
                                             stop=(jj == KB // 2 - 1))
                        heated = 0
                        for _ in range(units_per_jj):
                            done = True
                            for g in side:
                                try:
                                    next(g)
                                    done = False
                                    break
                                except StopIteration:
                                    continue
                            if done:
                                if heater is not None and heated < 2:
                                    heater.pull()
                                    heated += 1
                                else:
                                    break
                    for h in range(HPC):
                        rc = smpool.tile([1, QT], fp32, tag="rc")
                        nc.vector.reciprocal(rc[:], ot_ps[h][64:65, :])
                        bc = smpool.tile([64, QT], fp32, tag="bc")
                        nc.gpsimd.partition_broadcast(bc[:], rc[:])
                        if h == 0:
                            nc.vector.tensor_mul(
                                otn[0:64, t * QT:(t + 1) * QT],
                                ot_ps[h][0:64, :], bc[:])
                        else:
                            hi = smpool.tile([64, QT], bf16, tag="hi")
                            nc.vector.tensor_mul(hi[:], ot_ps[h][0:64, :],
                                                 bc[:])
                            nc.sync.dma_start(
                                otn[64:128, t * QT:(t + 1) * QT], hi[:])
                    if post_t_side is not None:
                        g = post_t_side(t)
                        if g is not None:
                            side.append(g)
                if heater is not None:
                    heater.finish()

            def drain(gens):
                for g in gens:
                    for _ in g:
                        pass

            # batch 0 projections + v' run standalone (they warm the clock)
            drain([gen_a(0)])
            drain([gen_vtr(0)])
            # attn(b0) with A(b1)+v'(b1) interleaved for clock-warming
            a1, v1 = gen_a(1), gen_vtr(1)
            emit_attention(0, [a1, v1], units_per_jj=5, heater=None)
            drain([a1, v1])
            # attn(b1) with C(b0) interleaved; C(b1) rowblocks released
            # per query-tile as their otn columns become trace-complete
            side_b1 = [gen_c(0)]

            def post_t(t):
                return gen_c(1, rows=range(t * NQT, (t + 1) * NQT))

            emit_attention(1, side_b1, units_per_jj=3, post_t_side=post_t,
                           heater=None)
            drain(side_b1)

    nc.compile()
    return nc


def _prep_inputs(x, attention_mask, w_attn, b_attn, w_proj):
    xT = np.ascontiguousarray(
        np.asarray(x, dtype=np.float32).reshape(BS, D).T).astype(BF16)
    maskt = np.ascontiguousarray(
        np.asarray(attention_mask, dtype=np.float32)
        .reshape(B, KB, 128).transpose(2, 0, 1).reshape(128, B * KB))
    w_attn = np.asarray(w_attn, dtype=np.float32)
    b_attn = np.asarray(b_attn, dtype=np.float32)
    w_proj = np.asarray(w_proj, dtype=np.float32)
    in_maps = []
    for c in range(NCORES):
        lo, hi = 2 * c * HD, (2 * c + 2) * HD
        in_maps.append({
            "xt": xT,
            "wq": np.ascontiguousarray(w_attn[:, lo:hi]).astype(BF16),
            "wk": np.ascontiguousarray(w_attn[:, D + lo: D + hi]).astype(BF16),
            "wv": np.ascontiguousarray(w_attn[:, 2 * D + lo: 2 * D + hi]).astype(BF16),
            "bq": np.ascontiguousarray(b_attn[lo:hi].reshape(128, 1)),
            "bk": np.ascontiguousarray(b_attn[D + lo: D + hi].reshape(128, 1)),
            "bv": np.ascontiguousarray(b_attn[2 * D + lo: 2 * D + hi].reshape(128, 1)),
            "wp": np.ascontiguousarray(w_proj[lo:hi, :]).astype(BF16),
            "maskt": maskt,
        })
    return in_maps


def _run(in_maps, trace=False, tmpdir=None):
    from concourse import bass_utils
    zero_mask = not np.any(in_maps[0]["maskt"])
    key = ("nc", zero_mask)
    if key not in _cache:
        _cache[key] = _build(zero_mask)
    return bass_utils.run_bass_kernel_spmd(
        _cache[key], in_maps, core_ids=list(range(NCORES)),
        trace=trace, tmpdir=tmpdir)


def kernel(x, attention_mask, w_attn, b_attn, w_proj, b_proj):
    in_maps = _prep_inputs(x, attention_mask, w_attn, b_attn, w_proj)
    res = _run(in_maps)
    out = np.zeros((BS, D), dtype=np.float32)
    for c in range(NCORES):
        out += res.results[c]["out"]
    out += np.asarray(b_proj, dtype=np.float32)[None, :]
    return out.reshape(B, S, D)


# revision 32
# speedup vs baseline: 1.8121x; 1.8121x over previous
"""Trainium2 Bass kernel for nn_Attention (B=2, S=2048, D=1024, H=16).

Sharding: tensor-parallel over heads. Each of the 8 cores owns 2 heads
(both batches): it computes q,k,v projections for its head columns, full
attention for its 4 (batch, head) pairs, and a partial output projection
(contraction over its 128 head-output columns). The host sums the 8
partials and adds b_proj.

Performance notes (measured on TRN2):
 - The PE HAM clock-gate only holds 2.4 GHz while "full" matmuls keep
   coming; the attention phase's half-array matmuls (K=64 scores,
   M=65 PV) alone leave it at 1.2 GHz. Stage A of the other batch and
   stage C are therefore emitted interleaved into the attention loops
   (>= ~1 full-array matmul / 1.3us keeps the clock warm).
 - exp on ScalarE has ~256ns fixed cost per op: score tiles are fused
   to [128,1024] (two keyblocks) when the additive mask is all-zero.
"""

import sys

sys.path.insert(0, "/opt/trn_rl_repo")

import numpy as np
import ml_dtypes

B, S, D, H, HD = 2, 2048, 1024, 16, 64
NCORES = 8
HPC = H // NCORES  # heads per core = 2
BS = B * S  # 4096
KB = S // 128  # key blocks per batch = 16
QT = 512  # query tile
NQT = S // QT  # query tiles per batch = 4
DC = D // 128  # contraction chunks = 8

BF16 = ml_dtypes.bfloat16

_cache = {}


def _build(zero_mask):
    import concourse.bass as bass
    import concourse.mybir as mybir
    import concourse.tile as tile
    from concourse import bacc
    from concourse.masks import make_identity

    fp32 = mybir.dt.float32
    bf16 = mybir.dt.bfloat16
    EXP = mybir.ActivationFunctionType.Exp

    nc = bacc.Bacc("TRN2", target_bir_lowering=False, debug=False,
                   num_devices=NCORES)

    xt_d = nc.dram_tensor("xt", [D, BS], bf16, kind="ExternalInput").ap()
    wq_d = nc.dram_tensor("wq", [D, 128], bf16, kind="ExternalInput").ap()
    wk_d = nc.dram_tensor("wk", [D, 128], bf16, kind="ExternalInput").ap()
    wv_d = nc.dram_tensor("wv", [D, 128], bf16, kind="ExternalInput").ap()
    bq_d = nc.dram_tensor("bq", [128, 1], fp32, kind="ExternalInput").ap()
    bk_d = nc.dram_tensor("bk", [128, 1], fp32, kind="ExternalInput").ap()
    bv_d = nc.dram_tensor("bv", [128, 1], fp32, kind="ExternalInput").ap()
    wp_d = nc.dram_tensor("wp", [128, D], bf16, kind="ExternalInput").ap()
    mk_d = nc.dram_tensor("maskt", [128, B * KB], fp32, kind="ExternalInput").ap()
    out_d = nc.dram_tensor("out", [BS, D], fp32, kind="ExternalOutput").ap()

    with tile.TileContext(nc) as tc:
        with (
            tc.tile_pool(name="const", bufs=1) as cpool,
            tc.tile_pool(name="xt", bufs=2 * DC) as xpool,
            tc.tile_pool(name="qkv", bufs=2) as qkvpool,
            tc.tile_pool(name="vp", bufs=2 * HPC * KB) as vppool,
            tc.tile_pool(name="pt", bufs=6) as ptpool,
            tc.tile_pool(name="otn", bufs=2) as otnpool,
            tc.tile_pool(name="small", bufs=2) as smpool,
            tc.tile_pool(name="cout", bufs=3) as coutpool,
            tc.tile_pool(name="ps_a", bufs=2, space="PSUM") as ps_a,
            tc.tile_pool(name="ps_st", bufs=2, space="PSUM") as ps_st,
            tc.tile_pool(name="ps_ot", bufs=2, space="PSUM") as ps_ot,
        ):
            # ---- constants ----
            wq_sb = cpool.tile([128, DC, 128], bf16)
            wk_sb = cpool.tile([128, DC, 128], bf16)
            wv_sb = cpool.tile([128, DC, 128], bf16)
            for w_sb, w_d in ((wq_sb, wq_d), (wk_sb, wk_d), (wv_sb, wv_d)):
                nc.sync.dma_start(w_sb[:], w_d.rearrange("(c p) m -> p c m", p=128))
            wp_sb = cpool.tile([128, D], bf16)
            nc.sync.dma_start(wp_sb[:], wp_d)
            bq_sb = cpool.tile([128, 1], fp32)
            bk_sb = cpool.tile([128, 1], fp32)
            bv_sb = cpool.tile([128, 1], fp32)
            for b_sb, b_d in ((bq_sb, bq_d), (bk_sb, bk_d), (bv_sb, bv_d)):
                nc.sync.dma_start(b_sb[:], b_d)
            mk_sb = cpool.tile([128, B * KB], fp32)
            nc.sync.dma_start(mk_sb[:], mk_d)
            ident = cpool.tile([128, 128], bf16)
            make_identity(nc, ident[:])

            qkvs = {}
            vps = {}
            otns = {}

            def gen_a(b):
                """Stage A for batch b: qT/kT/vT [128, S] (transposed,
                2 heads stacked). Yields after each emitted unit."""
                qT = qkvpool.tile([128, S], bf16, tag="qT", name=f"qT_{b}")
                kT = qkvpool.tile([128, S], bf16, tag="kT", name=f"kT_{b}")
                vT = qkvpool.tile([128, S], bf16, tag="vT", name=f"vT_{b}")
                qkvs[b] = (qT, kT, vT)
                for t in range(NQT):
                    xts = []
                    for c in range(DC):
                        xt = xpool.tile([128, QT], bf16, tag="xt", name="xt")
                        nc.sync.dma_start(
                            xt[:], xt_d[c * 128:(c + 1) * 128,
                                        b * S + t * QT: b * S + (t + 1) * QT])
                        xts.append(xt)
                    for (dst, w_sb, b_sb, tg) in (
                        (qT, wq_sb, bq_sb, "q"),
                        (kT, wk_sb, bk_sb, "k"),
                        (vT, wv_sb, bv_sb, "v"),
                    ):
                        a_ps = ps_a.tile([128, QT], fp32, tag="a",
                                         name=f"a_ps_{tg}")
                        for c in range(DC):
                            nc.tensor.matmul(a_ps[:], w_sb[:, c, :], xts[c][:],
                                             start=(c == 0), stop=(c == DC - 1))
                            yield
                        nc.vector.tensor_scalar_add(
                            dst[:, t * QT:(t + 1) * QT], a_ps[:], b_sb[:])
                        yield

            def gen_vtr(b):
                """v' tiles for batch b: [128 keys, 64 v + ones]."""
                vT = qkvs[b][2]
                for h in range(HPC):
                    for j in range(KB):
                        vtr_ps = ps_a.tile([128, 64], bf16, tag="a",
                                           name="vtr_ps")
                        nc.tensor.transpose(
                            vtr_ps[:],
                            vT[h * 64:(h + 1) * 64, j * 128:(j + 1) * 128],
                            ident[h * 64:(h + 1) * 64, h * 64:(h + 1) * 64])
                        vp = vppool.tile([128, 65], bf16, tag="vp",
                                         name=f"vp_{b}_{h}_{j}")
                        nc.vector.tensor_copy(vp[:, 0:64], vtr_ps[:])
                        nc.gpsimd.memset(vp[:, 64:65], 1.0)
                        vps[(b, h, j)] = vp
                        yield

            def gen_c(b, rows=None):
                """Stage C for batch b: partial out-projection."""
                otn = otns[b]
                for r in (range(S // 128) if rows is None else rows):
                    for n in range(D // QT):
                        c_ps = ps_a.tile([128, QT], fp32, tag="a", name="c_ps")
                        nc.tensor.matmul(c_ps[:],
                                         otn[:, r * 128:(r + 1) * 128],
                                         wp_sb[:, n * QT:(n + 1) * QT],
                                         start=True, stop=True)
                        co = coutpool.tile([128, QT], fp32, tag="co")
                        nc.vector.tensor_copy(co[:], c_ps[:])
                        nc.sync.dma_start(
                            out_d[b * S + r * 128: b * S + (r + 1) * 128,
                                  n * QT:(n + 1) * QT], co[:])
                        yield

            heat_sb = cpool.tile([128, QT], fp32)

            class Heater:
                """Dummy full-array accumulation chains into the ps_a ring
                (drained to a scratch sbuf tile). Keeps the PE HAM clock at
                2.4 GHz when no real full-array work is available. pull()
                emits one matmul; finish() closes any open chain."""

                def __init__(self):
                    self.tile = None
                    self.c = 0

                def pull(self):
                    if self.tile is None:
                        self.tile = ps_a.tile([128, QT], fp32, tag="a",
                                              name="heat_ps")
                        self.c = 0
                    nc.tensor.matmul(self.tile[:], wq_sb[:, self.c, :],
                                     wp_sb[:, 0:QT],
                                     start=(self.c == 0),
                                     stop=(self.c == DC - 1))
                    self.c += 1
                    if self.c == DC:
                        nc.vector.tensor_copy(heat_sb[:], self.tile[:])
                        self.tile = None

                def finish(self):
                    while self.tile is not None:
                        self.pull()

            def emit_attention(b, side, units_per_jj, post_t_side=None,
                               heater=None):
                """Attention for batch b; pulls `units_per_jj` units of
                side work after each keyblock-pair to keep the PE's HAM
                clock warm with full-array matmuls. post_t_side(t) may
                return an extra generator to append after query-tile t's
                normalization is emitted (trace-order-safe C(b1)).
                `heater` fills when real side work runs out."""
                qT, kT, vT = qkvs[b]
                otn = otnpool.tile([128, S], bf16, tag="otn", name=f"otn_{b}")
                otns[b] = otn
                for t in range(NQT):
                    ot_ps = [ps_ot.tile([65, QT], fp32, tag="ot",
                                        name=f"ot_ps_{b}_{t}_{h}")
                             for h in range(HPC)]
                    for jj in range(KB // 2):
                        j0, j1 = 2 * jj, 2 * jj + 1
                        st2s = [ps_st.tile([128, 2 * QT], fp32, tag="st",
                                           name=f"st_{h}")
                                for h in range(HPC)]
                        # both heads' K=64 score matmuls issued as an
                        # atomic pair: they occupy disjoint PE row-groups
                        # (rows 0-63 / 64-127) and run concurrently.
                        for ji, jx in ((0, j0), (1, j1)):
                            for h in range(HPC):
                                hs = slice(h * 64, (h + 1) * 64)
                                nc.tensor.matmul(
                                    st2s[h][:, ji * QT:(ji + 1) * QT],
                                    kT[hs, jx * 128:(jx + 1) * 128],
                                    qT[hs, t * QT:(t + 1) * QT],
                                    start=True, stop=True)
                        for h in range(HPC):
                            st2 = st2s[h]
                            pt2 = ptpool.tile([128, 2 * QT], bf16, tag="pt",
                                              name=f"pt_{h}")
                            if zero_mask:
                                nc.scalar.activation(pt2[:], st2[:], EXP,
                                                     bias=0.0, scale=0.125)
                            else:
                                for ji, jx in ((0, j0), (1, j1)):
                                    nc.scalar.activation(
                                        pt2[:, ji * QT:(ji + 1) * QT],
                                        st2[:, ji * QT:(ji + 1) * QT], EXP,
                                        bias=mk_sb[:, b * KB + jx:
                                                   b * KB + jx + 1],
                                        scale=0.125)
                            nc.tensor.matmul(ot_ps[h][:], vps[(b, h, j0)][:],
                                             pt2[:, 0:QT],
                                             start=(jj == 0), stop=False)
                            nc.tensor.matmul(ot_ps[h][:], vps[(b, h, j1)][:],
                                             pt2[:, QT:2 * QT],
                                             start=False,
                                             stop=(jj == KB // 2 - 1))
                        heated = 0
                        for _ in range(units_per_jj):
                            done = True
                            for g in side:
                                try:
                                    next(g)
                                    done = False
                                    break
                                except StopIteration:
                                    continue
                            if done:
                                if heater is not None and heated < 2:
                                    heater.pull()
                                    heated += 1
                                else:
                                    break
                    for h in range(HPC):
                        rc = smpool.tile([1, QT], fp32, tag="rc")
                        nc.vector.reciprocal(rc[:], ot_ps[h][64:65, :])
                        bc = smpool.tile([64, QT], fp32, tag="bc")
                        nc.gpsimd.partition_broadcast(bc[:], rc[:])
                        if h == 0:
                            nc.vector.tensor_mul(
                                otn[0:64, t * QT:(t + 1) * QT],
                                ot_ps[h][0:64, :], bc[:])
                        else:
                            hi = smpool.tile([64, QT], bf16, tag="hi")
                            nc.vector.tensor_mul(hi[:], ot_ps[h][0:64, :],
                                                 bc[:])
                            nc.sync.dma_start(
                                otn[64:128, t * QT:(t + 1) * QT], hi[:])
                    if post_t_side is not None:
                        g = post_t_side(t)
                        if g is not None:
                            side.append(g)
                if heater is not None:
                    heater.finish()

            def drain(gens):
                for g in gens:
                    for _ in g:
                        pass

            # batch 0 projections + v' run standalone (they warm the clock)
            drain([gen_a(0)])
            drain([gen_vtr(0)])
            # attn(b0) with A(b1)+v'(b1) interleaved for clock-warming
            a1, v1 = gen_a(1), gen_vtr(1)
            emit_attention(0, [a1, v1], units_per_jj=5, heater=None)
            drain([a1, v1])
            # attn(b1) with C(b0) interleaved; C(b1) rowblocks released
            # per query-tile as their otn columns become trace-complete
            side_b1 = [gen_c(0)]

            def post_t(t):
                return gen_c(1, rows=range(t * NQT, (t + 1) * NQT))

            emit_attention(1, side_b1, units_per_jj=3, post_t_side=post_t,
                           heater=None)
            drain(side_b1)

    nc.compile()
    return nc


def _prep_inputs(x, attention_mask, w_attn, b_attn, w_proj):
    xT = np.ascontiguousarray(
        np.asarray(x, dtype=np.float32).reshape(BS, D).T).astype(BF16)
    maskt = np.ascontiguousarray(
        np.asarray(attention_mask, dtype=np.float32)
        .reshape(B, KB, 128).transpose(2, 0, 1).reshape(128, B * KB))
    w_attn = np.asarray(w_attn, dtype=np.float32)
    b_attn = np.asarray(b_attn, dtype=np.float32)
    w_proj = np.asarray(w_proj, dtype=np.float32)
    in_maps = []
    for c in range(NCORES):
        lo, hi = 2 * c * HD, (2 * c + 2) * HD
        in_maps.append({
            "xt": xT,
            "wq": np.ascontiguousarray(w_attn[:, lo:hi]).astype(BF16),
            "wk": np.ascontiguousarray(w_attn[:, D + lo: D + hi]).astype(BF16),
            "wv": np.ascontiguousarray(w_attn[:, 2 * D + lo: 2 * D + hi]).astype(BF16),
            "bq": np.ascontiguousarray(b_attn[lo:hi].reshape(128, 1)),
            "bk": np.ascontiguousarray(b_attn[D + lo: D + hi].reshape(128, 1)),
            "bv": np.ascontiguousarray(b_attn[2 * D + lo: 2 * D + hi].reshape(128, 1)),
            "wp": np.ascontiguousarray(w_proj[lo:hi, :]).astype(BF16),
            "maskt": maskt,
        })
    return in_maps


def _run(in_maps, trace=False, tmpdir=None):
    from concourse import bass_utils
    zero_mask = not np.any(in_maps[0]["maskt"])
    key = ("nc", zero_mask)
    if key not in _cache:
        _cache[key] = _build(zero_mask)
    return bass_utils.run_bass_kernel_spmd(
        _cache[key], in_maps, core_ids=list(range(NCORES)),
        trace=trace, tmpdir=tmpdir)


def kernel(x, attention_mask, w_attn, b_attn, w_proj, b_proj):
    in_maps = _prep_inputs(x, attention_mask, w_attn, b_attn, w_proj)
    res = _run(in_maps)
    out = np.zeros((BS, D), dtype=np.float32)
    for c in range(NCORES):
        out += res.results[c]["out"]
    out += np.asarray(b_proj, dtype=np.float32)[None, :]
    return out.reshape(B, S, D)
